# revision 2
# baseline (speedup 1.0000x reference)
"""ContextualConv2d Trainium2 kernel (v2: kh-packed K=96 col-tiled fp16).

Problem: grouped 3x3 conv (N=32, 128ci -> 256co, groups=4, 56x56, pad 1)
plus per-(batch,channel) context bias: out = conv(x, w) + (c @ cwT)[n,co]
+ bias[co].

Sharding (8 cores): core = (group-pair gp in {0,1}) x (batch quarter q in
{0..3}). Each core computes 8 images x 128 out-channels (2 groups of 64).

Per-core compute scheme:
  - x is uploaded in a host-packed "3-band" layout: partition p = kh*32+ci
    holds padded-x rows shifted by kh. This lifts the kh taps into the
    matmul contraction dim: one matmul per kw contracts (ci=32) x (kh=3)
    = K=96, accumulating 3 matmuls per output tile instead of 9.
  - M=64 (one group's out-channels); the two groups run as 2x column
    tiles of the PE array (tile_position (0,0) and (0,64)), which the PE
    can stream concurrently (independent weight cols + rhs xbuses).
  - fp16 operands (fp32 PSUM accumulate) halve DMA and SBUF traffic;
    output is written fp16 and widened to fp32 on the host.
  - epilogue fuses the context/bias add (per-partition scalar) with the
    PSUM->SBUF copy, alternating DVE/ACT engines per half-tile.
"""

import numpy as np

from concourse import bass, mybir, tile
from concourse.vector_clock import ScopedClock
from concourse.bass_utils import run_bass_kernel_spmd

N, CIN, H, W = 32, 128, 56, 56
COUT, KH, KW = 256, 3, 3
GROUPS = 4
CDIM = 64
WP = W + 2            # padded width (58)
ROWS = 8              # output rows per n-tile
NT = H // ROWS        # 7 n-tiles per image
NFREE = ROWS * W      # 448 <= 512 fp32 PSUM bank limit
N_CORES = 8
IMGS = N // 4         # 8 images per core
CO = COUT // 2        # 128 output channels per core (2 groups)
KPACK = KH * 32       # 96 contraction rows (kh x ci)


class _TC(tile.TileContext):
    """This container's walrus accepts only one sem wait on a Drain
    (CTRL) instruction; TileContext's tail drain aggregates one wait per
    outstanding semaphore. Split them across sequential drains."""

    def _drain_and_barrier(self, tick_clock, wait_clock):
        drain_inst = self.nc.sync.drain()
        wait_clock.add_sem_waits(
            drain_inst.ins, ScopedClock({None: tick_clock.global_clock})
        )
        si = drain_inst.ins.sync_info
        if si is not None and len(si.on_wait) > 1:
            waits = list(si.on_wait)
            si.on_wait.clear()
            si.on_wait.append(waits[0])
            for w in waits[1:]:
                d2 = self.nc.sync.drain()
                d2.ins.sync_info = mybir.SyncInfo(on_wait=[w], on_update=[])
        self.nc.all_engine_barrier()
        assert self.sems is not None
        popped = self.nc._tile_sem_poison_stack.pop()
        assert popped is self._sem_poison
        self.nc.clear_and_free_semaphores(list(self.sems.allocated().values()))
        self.nc.all_engine_barrier()


_ws_ctr = [0]


def _split_waits(nc):
    """Walrus here caps sem waits at one per instruction; hoist extras
    onto injected same-engine NoOps placed just before the owner."""
    for fn in nc.m.functions:
        for blk in fn.blocks:
            insts = blk.instructions
            out = []
            changed = False
            for inst in insts:
                si = getattr(inst, "sync_info", None)
                if si is not None and si.on_wait and len(si.on_wait) > 1:
                    waits = list(si.on_wait)
                    for w in waits[:-1]:
                        _ws_ctr[0] += 1
                        out.append(
                            mybir.InstNoOp(
                                name=f"WSNOP-{_ws_ctr[0]}",
                                engine=inst.engine,
                                ins=[],
                                outs=[],
                                sync_info=mybir.SyncInfo(on_wait=[w], on_update=[]),
                                debug=inst.debug,
                            )
                        )
                        changed = True
                    si.on_wait.clear()
                    si.on_wait.append(waits[-1])
                out.append(inst)
            if changed:
                insts.clear()
                insts.extend(out)
    return nc


def build_program(loop_n: int = 0):
    """loop_n > 0 builds a benchmark variant: the conv body repeats
    loop_n times inside a hardware For_i so device time dominates the
    (RPC/transfer-heavy) wall clock. loop_n=0 is the production kernel."""
    f32 = mybir.dt.float32
    f32r = mybir.dt.float32r
    f16 = mybir.dt.float16
    nc = bass.Bass("TRN2", target_bir_lowering=False, debug=False)
    xs3 = nc.declare_dram_parameter("xs3", [IMGS, KPACK, 2, H, WP], f16, isOutput=False)
    wb = nc.declare_dram_parameter("wb", [KPACK, 6, 64], f16, isOutput=False)
    cwb = nc.declare_dram_parameter("cwb", [CDIM + 1, CO], f32r, isOutput=False)
    cb = nc.declare_dram_parameter("cb", [CDIM + 1, IMGS], f32r, isOutput=False)
    y = nc.declare_dram_parameter("y", [IMGS, CO, H, W], f16, isOutput=True)

    with _TC(nc) as tc:
        with (
            tc.tile_pool(name="wp", bufs=1) as wpool,
            tc.tile_pool(name="xp", bufs=3) as xpool,
            tc.tile_pool(name="op", bufs=4) as opool,
            tc.tile_pool(name="psp", bufs=6, space="PSUM") as pspool,
            tc.tile_pool(name="psc", bufs=1, space="PSUM") as pscpool,
        ):
            wt = wpool.tile([KPACK, 6, 64], f16)
            nc.sync.dma_start(wt[:], wb[:])
            cwbt = wpool.tile([CDIM + 1, CO], f32r)
            nc.sync.dma_start(cwbt[:], cwb[:])
            cbt = wpool.tile([CDIM + 1, IMGS], f32r)
            nc.sync.dma_start(cbt[:], cb[:])

            # bctx[co, n] = sum_d c_weight[co,d] c[n,d] + bias[co]
            psc = pscpool.tile([CO, IMGS], f32)
            nc.tensor.matmul(psc[:, :], cwbt[:], cbt[:], start=True, stop=True)
            bctx = wpool.tile([CO, IMGS], f32)
            nc.vector.tensor_copy(bctx[:], psc[:, :])

            def conv_body():
                for i in range(IMGS):
                    xt = xpool.tile([KPACK, 2, H, WP], f16)
                    nc.sync.dma_start(xt[:], xs3[i])
                    ot = opool.tile([CO, H * W], f16, name=f"ot{i}", tag="ot")
                    for t in range(NT):
                        ps = pspool.tile([CO, NFREE], f32, name=f"ps{i}_{t}", tag="ps")
                        for kw in range(3):
                            nc.tensor.matmul(
                                ps[0:64, :],
                                wt[:, kw, :],
                                xt[:, 0, t * ROWS : t * ROWS + ROWS, kw : kw + W],
                                start=(kw == 0),
                                stop=(kw == 2),
                                tile_position=(0, 0),
                            )
                            nc.tensor.matmul(
                                ps[64:128, :],
                                wt[:, 3 + kw, :],
                                xt[:, 1, t * ROWS : t * ROWS + ROWS, kw : kw + W],
                                start=(kw == 0),
                                stop=(kw == 2),
                                tile_position=(0, 64),
                            )
                        oA = ot[0:64, t * NFREE : (t + 1) * NFREE]
                        oB = ot[64:128, t * NFREE : (t + 1) * NFREE]
                        if t % 2 == 0:
                            nc.vector.tensor_scalar_add(oA, ps[0:64, :], bctx[0:64, i : i + 1])
                            nc.scalar.activation(
                                oB, ps[64:128, :], mybir.ActivationFunctionType.Identity,
                                bias=bctx[64:128, i : i + 1],
                            )
                        else:
                            nc.scalar.activation(
                                oA, ps[0:64, :], mybir.ActivationFunctionType.Identity,
                                bias=bctx[0:64, i : i + 1],
                            )
                            nc.vector.tensor_scalar_add(oB, ps[64:128, :], bctx[64:128, i : i + 1])
                    nc.sync.dma_start(y[i].rearrange("c h w -> c (h w)"), ot[:])

            if loop_n > 0:
                with tc.For_i(0, loop_n, 1, hint_engines=(mybir.EngineType.PE,)):
                    conv_body()
            else:
                conv_body()
    _split_waits(nc)
    return nc


_prog_cache = {}


def _get_program():
    if "nc" not in _prog_cache:
        _prog_cache["nc"] = build_program()
    return _prog_cache["nc"]


def _shard_inputs(x, c, weight, bias, c_weight):
    """Build the per-core input dicts (pure layout prep, no math)."""
    xpad = np.zeros((N, CIN, H + 2, WP), np.float16)
    xpad[:, :, 1 : H + 1, 1 : W + 1] = x.astype(np.float16)

    # Weights: wb[gp][kh*32+ci, g*3+kw, co] = weight[128gp+64g+co, ci, kh, kw]
    w16 = weight.astype(np.float16)  # [256, 32, 3, 3]
    wbs = []
    cwbs = []
    for gp in range(2):
        wsl = w16[CO * gp : CO * gp + CO]             # [128, 32, 3, 3]
        # -> [kh, ci, g, kw, co]
        blk = wsl.reshape(2, 64, 32, 3, 3).transpose(3, 2, 0, 4, 1)
        # blk[kh, ci, g, kw, co]; flatten to [96, 6, 64]
        wbs.append(np.ascontiguousarray(blk.reshape(KPACK, 6, 64)))

        cwbv = np.empty((CDIM + 1, CO), np.float32)
        cwbv[:CDIM] = c_weight[CO * gp : CO * gp + CO].T
        cwbv[CDIM] = bias[CO * gp : CO * gp + CO]
        cwbs.append(cwbv)

    # x bands: xs3[i, kh*32+ci, g, r, c] = xpad[img, 64gp+32g+ci, r+kh, c]
    xs3s = []
    for gp in range(2):
        sub = xpad[:, 64 * gp : 64 * gp + 64]         # [32, 64, 58, 58]
        sub = sub.reshape(N, 2, 32, H + 2, WP)         # [n, g, ci, hp, wp]
        bands = np.stack(
            [sub[:, :, :, kh : kh + H, :] for kh in range(KH)], axis=1
        )                                              # [n, kh, g, ci, 56, 58]
        xs3s.append(np.ascontiguousarray(
            bands.transpose(0, 1, 3, 2, 4, 5).reshape(N, KPACK, 2, H, WP)
        ))

    in_maps = []
    for core in range(N_CORES):
        gp, q = divmod(core, 4)
        cbv = np.empty((CDIM + 1, IMGS), np.float32)
        cbv[:CDIM] = c[IMGS * q : IMGS * q + IMGS].T
        cbv[CDIM] = 1.0
        in_maps.append(
            {
                "xs3": np.ascontiguousarray(xs3s[gp][IMGS * q : IMGS * q + IMGS]),
                "wb": wbs[gp],
                "cwb": cwbs[gp],
                "cb": cbv,
            }
        )
    return in_maps


def kernel(x, c, weight, bias, c_weight):
    x = np.asarray(x, np.float32)
    c = np.asarray(c, np.float32)
    weight = np.asarray(weight, np.float32)
    bias = np.asarray(bias, np.float32)
    c_weight = np.asarray(c_weight, np.float32)

    nc = _get_program()
    in_maps = _shard_inputs(x, c, weight, bias, c_weight)
    res = run_bass_kernel_spmd(nc, in_maps, list(range(N_CORES)), trace=False)

    out = np.empty((N, COUT, H, W), np.float32)
    for core in range(N_CORES):
        gp, q = divmod(core, 4)
        out[IMGS * q : IMGS * q + IMGS, CO * gp : CO * gp + CO] = (
            res.results[core]["y"].astype(np.float32)
        )
    return out


# revision 5
# speedup vs baseline: 1.9106x; 1.9106x over previous
"""ContextualConv2d Trainium2 kernel (v2: kh-packed K=96 col-tiled fp16).

Problem: grouped 3x3 conv (N=32, 128ci -> 256co, groups=4, 56x56, pad 1)
plus per-(batch,channel) context bias: out = conv(x, w) + (c @ cwT)[n,co]
+ bias[co].

Sharding (8 cores): core = (group-pair gp in {0,1}) x (batch quarter q in
{0..3}). Each core computes 8 images x 128 out-channels (2 groups of 64).

Per-core compute scheme:
  - x is uploaded in a host-packed "3-band" layout: partition p = kh*32+ci
    holds padded-x rows shifted by kh. This lifts the kh taps into the
    matmul contraction dim: one matmul per kw contracts (ci=32) x (kh=3)
    = K=96, accumulating 3 matmuls per output tile instead of 9.
  - M=64 (one group's out-channels); the two groups run as 2x column
    tiles of the PE array (tile_position (0,0) and (0,64)), which the PE
    can stream concurrently (independent weight cols + rhs xbuses).
  - fp16 operands (fp32 PSUM accumulate) halve DMA and SBUF traffic;
    output is written fp16 and widened to fp32 on the host.
  - epilogue fuses the context/bias add (per-partition scalar) with the
    PSUM->SBUF copy, alternating DVE/ACT engines per half-tile.
"""

import numpy as np

from concourse import bass, mybir, tile
from concourse.vector_clock import ScopedClock
from concourse.bass_utils import run_bass_kernel_spmd

N, CIN, H, W = 32, 128, 56, 56
COUT, KH, KW = 256, 3, 3
GROUPS = 4
CDIM = 64
WP = W + 2            # padded width (58)
ROWS = 8              # output rows per n-tile
NT = H // ROWS        # 7 n-tiles per image
NFREE = ROWS * W      # 448 <= 512 fp32 PSUM bank limit
N_CORES = 8
IMGS = N // 4         # 8 images per core
CO = COUT // 2        # 128 output channels per core (2 groups)
KPACK = KH * 32       # 96 contraction rows (kh x ci)


class _TC(tile.TileContext):
    """This container's walrus accepts only one sem wait on a Drain
    (CTRL) instruction; TileContext's tail drain aggregates one wait per
    outstanding semaphore. Split them across sequential drains."""

    def _drain_and_barrier(self, tick_clock, wait_clock):
        drain_inst = self.nc.sync.drain()
        wait_clock.add_sem_waits(
            drain_inst.ins, ScopedClock({None: tick_clock.global_clock})
        )
        si = drain_inst.ins.sync_info
        if si is not None and len(si.on_wait) > 1:
            waits = list(si.on_wait)
            si.on_wait.clear()
            si.on_wait.append(waits[0])
            for w in waits[1:]:
                d2 = self.nc.sync.drain()
                d2.ins.sync_info = mybir.SyncInfo(on_wait=[w], on_update=[])
        self.nc.all_engine_barrier()
        assert self.sems is not None
        popped = self.nc._tile_sem_poison_stack.pop()
        assert popped is self._sem_poison
        self.nc.clear_and_free_semaphores(list(self.sems.allocated().values()))
        self.nc.all_engine_barrier()


_ws_ctr = [0]


def _split_waits(nc):
    """Walrus here caps sem waits at one per instruction; hoist extras
    onto injected same-engine NoOps placed just before the owner."""
    for fn in nc.m.functions:
        for blk in fn.blocks:
            insts = blk.instructions
            out = []
            changed = False
            for inst in insts:
                si = getattr(inst, "sync_info", None)
                if si is not None and si.on_wait and len(si.on_wait) > 1:
                    waits = list(si.on_wait)
                    for w in waits[:-1]:
                        _ws_ctr[0] += 1
                        out.append(
                            mybir.InstNoOp(
                                name=f"WSNOP-{_ws_ctr[0]}",
                                engine=inst.engine,
                                ins=[],
                                outs=[],
                                sync_info=mybir.SyncInfo(on_wait=[w], on_update=[]),
                                debug=inst.debug,
                            )
                        )
                        changed = True
                    si.on_wait.clear()
                    si.on_wait.append(waits[-1])
                out.append(inst)
            if changed:
                insts.clear()
                insts.extend(out)
    return nc


def build_program(loop_n: int = 0):
    """loop_n > 0 builds a benchmark variant: the conv body repeats
    loop_n times inside a hardware For_i so device time dominates the
    (RPC/transfer-heavy) wall clock. loop_n=0 is the production kernel."""
    f32 = mybir.dt.float32
    f32r = mybir.dt.float32r
    f16 = mybir.dt.float16
    nc = bass.Bass("TRN2", target_bir_lowering=False, debug=False)
    xs3 = nc.declare_dram_parameter("xs3", [IMGS, KPACK, 2, H, WP], f16, isOutput=False)
    wb = nc.declare_dram_parameter("wb", [KPACK, 6, 64], f16, isOutput=False)
    cwb = nc.declare_dram_parameter("cwb", [CDIM + 1, CO], f32r, isOutput=False)
    cb = nc.declare_dram_parameter("cb", [CDIM + 1, IMGS], f32r, isOutput=False)
    y = nc.declare_dram_parameter("y", [IMGS, CO, H, W], f16, isOutput=True)

    with _TC(nc) as tc:
        with (
            tc.tile_pool(name="wp", bufs=1) as wpool,
            tc.tile_pool(name="xp", bufs=3) as xpool,
            tc.tile_pool(name="op", bufs=4) as opool,
            tc.tile_pool(name="psp", bufs=6, space="PSUM") as pspool,
            tc.tile_pool(name="psc", bufs=1, space="PSUM") as pscpool,
        ):
            wt = wpool.tile([KPACK, 6, 64], f16)
            nc.sync.dma_start(wt[:], wb[:])
            cwbt = wpool.tile([CDIM + 1, CO], f32r)
            nc.sync.dma_start(cwbt[:], cwb[:])
            cbt = wpool.tile([CDIM + 1, IMGS], f32r)
            nc.sync.dma_start(cbt[:], cb[:])

            # bctx[co, n] = sum_d c_weight[co,d] c[n,d] + bias[co]
            psc = pscpool.tile([CO, IMGS], f32)
            nc.tensor.matmul(psc[:, :], cwbt[:], cbt[:], start=True, stop=True)
            bctx = wpool.tile([CO, IMGS], f32)
            nc.vector.tensor_copy(bctx[:], psc[:, :])

            def conv_body():
                for i in range(IMGS):
                    xdma = nc.sync if i % 2 == 0 else nc.scalar
                    ydma = nc.scalar if i % 2 == 0 else nc.sync
                    xt = xpool.tile([KPACK, 2, H, WP], f16)
                    xdma.dma_start(xt[:], xs3[i])
                    ot = opool.tile([CO, H * W], f16, name=f"ot{i}", tag="ot")
                    for t in range(NT):
                        ps = pspool.tile([CO, NFREE], f32, name=f"ps{i}_{t}", tag="ps")
                        for kw in range(3):
                            nc.tensor.matmul(
                                ps[0:64, :],
                                wt[:, kw, :],
                                xt[:, 0, t * ROWS : t * ROWS + ROWS, kw : kw + W],
                                start=(kw == 0),
                                stop=(kw == 2),
                                tile_position=(0, 0),
                            )
                            nc.tensor.matmul(
                                ps[64:128, :],
                                wt[:, 3 + kw, :],
                                xt[:, 1, t * ROWS : t * ROWS + ROWS, kw : kw + W],
                                start=(kw == 0),
                                stop=(kw == 2),
                                tile_position=(0, 64),
                            )
                        o = ot[:, t * NFREE : (t + 1) * NFREE]
                        if (i * NT + t) % 2 == 0:
                            nc.vector.tensor_scalar_add(o, ps[:, :], bctx[:, i : i + 1])
                        else:
                            nc.scalar.activation(
                                o, ps[:, :], mybir.ActivationFunctionType.Identity,
                                bias=bctx[:, i : i + 1],
                            )
                    ydma.dma_start(y[i].rearrange("c h w -> c (h w)"), ot[:])

            if loop_n > 0:
                with tc.For_i(0, loop_n, 1, hint_engines=(mybir.EngineType.PE,)):
                    conv_body()
            else:
                conv_body()
    _split_waits(nc)
    return nc


_prog_cache = {}


def _get_program():
    if "nc" not in _prog_cache:
        _prog_cache["nc"] = build_program()
    return _prog_cache["nc"]


def _shard_inputs(x, c, weight, bias, c_weight):
    """Build the per-core input dicts (pure layout prep, no math)."""
    xpad = np.zeros((N, CIN, H + 2, WP), np.float16)
    xpad[:, :, 1 : H + 1, 1 : W + 1] = x.astype(np.float16)

    # Weights: wb[gp][kh*32+ci, g*3+kw, co] = weight[128gp+64g+co, ci, kh, kw]
    w16 = weight.astype(np.float16)  # [256, 32, 3, 3]
    wbs = []
    cwbs = []
    for gp in range(2):
        wsl = w16[CO * gp : CO * gp + CO]             # [128, 32, 3, 3]
        # -> [kh, ci, g, kw, co]
        blk = wsl.reshape(2, 64, 32, 3, 3).transpose(3, 2, 0, 4, 1)
        # blk[kh, ci, g, kw, co]; flatten to [96, 6, 64]
        wbs.append(np.ascontiguousarray(blk.reshape(KPACK, 6, 64)))

        cwbv = np.empty((CDIM + 1, CO), np.float32)
        cwbv[:CDIM] = c_weight[CO * gp : CO * gp + CO].T
        cwbv[CDIM] = bias[CO * gp : CO * gp + CO]
        cwbs.append(cwbv)

    # x bands: xs3[i, kh*32+ci, g, r, c] = xpad[img, 64gp+32g+ci, r+kh, c]
    xs3s = []
    for gp in range(2):
        sub = xpad[:, 64 * gp : 64 * gp + 64]         # [32, 64, 58, 58]
        sub = sub.reshape(N, 2, 32, H + 2, WP)         # [n, g, ci, hp, wp]
        bands = np.stack(
            [sub[:, :, :, kh : kh + H, :] for kh in range(KH)], axis=1
        )                                              # [n, kh, g, ci, 56, 58]
        xs3s.append(np.ascontiguousarray(
            bands.transpose(0, 1, 3, 2, 4, 5).reshape(N, KPACK, 2, H, WP)
        ))

    in_maps = []
    for core in range(N_CORES):
        gp, q = divmod(core, 4)
        cbv = np.empty((CDIM + 1, IMGS), np.float32)
        cbv[:CDIM] = c[IMGS * q : IMGS * q + IMGS].T
        cbv[CDIM] = 1.0
        in_maps.append(
            {
                "xs3": np.ascontiguousarray(xs3s[gp][IMGS * q : IMGS * q + IMGS]),
                "wb": wbs[gp],
                "cwb": cwbs[gp],
                "cb": cbv,
            }
        )
    return in_maps


def kernel(x, c, weight, bias, c_weight):
    x = np.asarray(x, np.float32)
    c = np.asarray(c, np.float32)
    weight = np.asarray(weight, np.float32)
    bias = np.asarray(bias, np.float32)
    c_weight = np.asarray(c_weight, np.float32)

    nc = _get_program()
    in_maps = _shard_inputs(x, c, weight, bias, c_weight)
    res = run_bass_kernel_spmd(nc, in_maps, list(range(N_CORES)), trace=False)

    out = np.empty((N, COUT, H, W), np.float32)
    for core in range(N_CORES):
        gp, q = divmod(core, 4)
        out[IMGS * q : IMGS * q + IMGS, CO * gp : CO * gp + CO] = (
            res.results[core]["y"].astype(np.float32)
        )
    return out
